# revision 1
# baseline (speedup 1.0000x reference)
"""GridToStation Trainium2 kernel, v2.

Pipeline (per core, SPMD x8):
  - Host: exact reference index math (f32). Stations sorted by ix0 and split
    into 8 equal chunks of 2048 -> perfect load balance. Within a core,
    stations sort by iy0 and cut into 6 fixed-size bands ([3,3,3,3,3,1]
    tiles of 128). Each band's grid rows are copied into a fixed-offset
    32768-row slab of the core's table (bf16, (H,W,C) layout, per-core
    column window of WTBL columns), so the per-band gather uses int16
    indices relative to a compile-time-constant slab base.
  - Host table rows are 4-corner blocks [v00|v01|v10|v11] (1024 bf16 =
    2KB per (iy0,ix0) cell), so each station needs ONE gather descriptor.
    Per band, ONE dma_gather (InstDMAGatherAnt) fetches all its stations,
    amortizing the SWDGE descriptor-gen launch on Pool.
  - Bilinear combine fused into the PE transpose: per tile, DVE builds 4
    diagonal matrices diag(c_j) (tensor_scalar_mul of the identity, 4x DVE
    mode), and 8 PE matmuls x^T[chunk] += v_j_chunk^T @ diag(c_j)
    accumulate the weighted transpose directly in PSUM (f32).
  - PSUM -> SBUF bf16 copies (DVE/ACT split), 2-layer MLP in bf16 on PE,
    Gelu(+bias) on ACT, y written back as bf16 (b1/b2==0 fast variants
    compiled adaptively); chunked output DMA; host upcasts/permutes.
"""

import os

import numpy as np

B, C, H, W, N = 1, 256, 721, 1440, 16384
NCORES = 8
NPC = N // NCORES  # 2048 stations per core
WTBL = 192  # per-core column window
SLABROWS = 32768  # table rows per band slab (int16-addressable)
BANDS_T = [1, 3, 3, 3, 3, 3]  # tiles per band (fixed); small first band starts the pipeline early
NBANDS = len(BANDS_T)
T = NPC // 128  # 16 tiles
TROWS = NBANDS * SLABROWS
NP = NPC
CB = 4 * C  # 4-corner block elems per table row
MAX_IDX = SLABROWS - 2

_PROG_CACHE = {}

LAST_RUN_INFO = {}


def _f32(x):
    return np.float32(x)


def _host_route(station_coords):
    lat = np.asarray(station_coords[0, :, 0], dtype=np.float32)
    lon = np.asarray(station_coords[0, :, 1], dtype=np.float32)
    lat_n = lat / _f32(90.0)
    lon_n = lon / _f32(180.0)
    ix = np.clip((lon_n + _f32(1.0)) * _f32(0.5) * _f32(W - 1), _f32(0.0), _f32(W - 1))
    iy = np.clip((lat_n + _f32(1.0)) * _f32(0.5) * _f32(H - 1), _f32(0.0), _f32(H - 1))
    ix0f = np.floor(ix)
    iy0f = np.floor(iy)
    wx = (ix - ix0f).astype(np.float32)
    wy = (iy - iy0f).astype(np.float32)
    ix0 = ix0f.astype(np.int32)
    iy0 = iy0f.astype(np.int32)
    iy1 = np.minimum(iy0 + 1, H - 1)
    one = _f32(1.0)
    cjs = (
        (one - wx) * (one - wy),
        wx * (one - wy),
        (one - wx) * wy,
        wx * wy,
    )
    return ix0, iy0, iy1, cjs


def _build_program(b1z=False, b2z=False):
    import concourse.bacc as bacc
    import concourse.bass as bass
    import concourse.mybir as mybir
    from concourse.tile import TileContext

    f32 = mybir.dt.float32
    bf16 = mybir.dt.bfloat16
    i16 = mybir.dt.int16
    AF = mybir.ActivationFunctionType
    ALU = mybir.AluOpType

    nc = bacc.Bacc("TRN2", target_bir_lowering=False, debug=False, num_swdge_queues=2)

    tbl = nc.dram_tensor("tbl", [TROWS, CB], bf16, kind="ExternalInput")
    # int16 idx, [128, sum over bands of kb*128/16] packed per band
    idx_cols = [kb * 128 // 16 for kb in BANDS_T]
    idx_off = np.cumsum([0] + idx_cols).tolist()
    idx = nc.dram_tensor("idx", [128, idx_off[-1]], i16, kind="ExternalInput")
    cof = nc.dram_tensor("cof", [128, 4 * T], bf16, kind="ExternalInput")
    w1 = nc.dram_tensor("w1t", [C, C], bf16, kind="ExternalInput")
    w2 = nc.dram_tensor("w2t", [C, C], bf16, kind="ExternalInput")
    bia = nc.dram_tensor("bia", [128, 4], f32, kind="ExternalInput")
    idn = nc.dram_tensor("idn", [128, 512], bf16, kind="ExternalInput")
    out = nc.dram_tensor("out", [2, 128, NP], bf16, kind="ExternalOutput")

    KBMAX = max(BANDS_T)

    with TileContext(nc) as tc:
        with (
            tc.tile_pool(name="const", bufs=1) as cpool,
            tc.tile_pool(name="gat", bufs=6) as gpool,
            tc.tile_pool(name="dg", bufs=16) as dpool,
            tc.tile_pool(name="xs", bufs=4) as xpool,
            tc.tile_pool(name="hs", bufs=4) as hpool,
            tc.tile_pool(name="px", bufs=4, space="PSUM") as pxp,
            tc.tile_pool(name="ph", bufs=1, space="PSUM") as php,
            tc.tile_pool(name="py", bufs=1, space="PSUM") as pyp,
        ):
            idx_sb = cpool.tile([128, idx_off[-1]], i16)
            nc.sync.dma_start(out=idx_sb[:], in_=idx[:])
            cof_sb = cpool.tile([128, 4 * T], bf16)
            nc.sync.dma_start(out=cof_sb[:], in_=cof[:])
            idn_sb = cpool.tile([128, 512], bf16)
            nc.sync.dma_start(out=idn_sb[:], in_=idn[:])
            bia_sb = cpool.tile([128, 4], f32)
            nc.sync.dma_start(out=bia_sb[:], in_=bia[:])
            w1_sb = cpool.tile([128, 2 * C], bf16)
            nc.scalar.dma_start(out=w1_sb[:, 0:C], in_=w1[0:128, :])
            nc.scalar.dma_start(out=w1_sb[:, C : 2 * C], in_=w1[128:256, :])
            w2_sb = cpool.tile([128, 2 * C], bf16)
            nc.scalar.dma_start(out=w2_sb[:, 0:C], in_=w2[0:128, :])
            nc.scalar.dma_start(out=w2_sb[:, C : 2 * C], in_=w2[128:256, :])
            out_sb = cpool.tile([128, 2 * NP], bf16)

            # band gathers (Pool engine runs ONLY these; mlp library stays
            # resident)
            gts = []
            tbl_ap = tbl[:]
            band_cut = []
            acc = 0
            for kb in BANDS_T:
                band_cut.append((acc, acc + kb))
                acc += kb
            nregs = {kb: nc.gpsimd.to_reg(kb * 128) for kb in sorted(set(BANDS_T))}
            for b, kb in enumerate(BANDS_T):
                gt_t = gpool.tile([128, KBMAX, CB], bf16, name="gt")
                in_ap = bass.AP(
                    tbl_ap.tensor,
                    b * SLABROWS * CB,
                    [[CB, SLABROWS], [1, CB]],
                )
                num_idxs = kb * 128
                nc.gpsimd.dma_gather(
                    out_ap=gt_t[:, 0:kb, :],
                    in_ap=in_ap,
                    idxs_ap=idx_sb[:, idx_off[b] : idx_off[b + 1]],
                    num_idxs=num_idxs,
                    num_idxs_reg=nregs[kb],
                    elem_size=CB,
                    elem_step=CB,
                    queue_num=b % 2,
                )
                gts.append(gt_t)

            def tile_src(t):
                for b, (t0, t1) in enumerate(band_cut):
                    if t < t1:
                        return gts[b], t - t0
                raise AssertionError(t)

            gelu_f = AF.Identity if os.environ.get("GRIDSTN_NOGELU") else AF.Gelu

            for qd in range(T // 4):
                xs = xpool.tile([128, 1024], bf16, name="xs")
                for qp in range(2):
                    pr = qd * 2 + qp
                    px = pxp.tile([128, 512], f32, name="px")
                    for tt in range(2):
                        t = pr * 2 + tt
                        gt_t, tl = tile_src(t)
                        dg = dpool.tile([128, 512], bf16, name="dg")
                        dga = dg[:]
                        cofv = cof_sb[:]
                        nc.vector.tensor_tensor(
                            out=bass.AP(
                                dga.tensor, dga.offset,
                                [dga.ap[0], [128, 4], [1, 128]],
                            ),
                            in0=bass.AP(
                                idn_sb[:].tensor, idn_sb[:].offset,
                                [idn_sb[:].ap[0], [128, 4], [1, 128]],
                            ),
                            in1=bass.AP(
                                cofv.tensor, cofv.offset + 4 * t,
                                [cofv.ap[0], [1, 4], [0, 128]],
                            ),
                            op=ALU.mult,
                        )
                        for ch in range(2):
                            for j in range(4):
                                xo = j * 256 + ch * 128
                                nc.tensor.matmul(
                                    out=px[:, ch * 256 + tt * 128 : ch * 256 + tt * 128 + 128],
                                    lhsT=gt_t[:, tl, xo : xo + 128],
                                    rhs=dg[:, j * 128 : (j + 1) * 128],
                                    start=(j == 0),
                                    stop=(j == 3),
                                )
                    if qp == 1:
                        nc.scalar.activation(
                            out=xs[:, 512:1024], in_=px[:], func=AF.Copy
                        )
                    else:
                        nc.vector.tensor_copy(xs[:, 0:512], px[:])
                ph = php.tile([128, 1024], f32, name="ph")
                for m in range(2):
                    for k in range(2):
                        rhs = bass.AP(
                            xs[:].tensor,
                            xs[:].offset + k * 256,
                            [xs[:].ap[0], [512, 2], [1, 256]],
                        )
                        nc.tensor.matmul(
                            out=ph[:, m * 512 : (m + 1) * 512],
                            lhsT=w1_sb[:, k * C + m * 128 : k * C + (m + 1) * 128],
                            rhs=rhs,
                            start=(k == 0),
                            stop=(k == 1),
                        )
                hs = hpool.tile([128, 1024], bf16, name="hs")
                if b1z:
                    nc.scalar.activation(out=hs[:], in_=ph[:], func=gelu_f)
                else:
                    for m in range(2):
                        nc.scalar.activation(
                            out=hs[:, m * 512 : (m + 1) * 512],
                            in_=ph[:, m * 512 : (m + 1) * 512],
                            func=gelu_f,
                            bias=bia_sb[:, m : m + 1],
                            scale=1.0,
                        )
                py = pyp.tile([128, 1024], f32, name="py")
                for m in range(2):
                    for k in range(2):
                        nc.tensor.matmul(
                            out=py[:, m * 512 : (m + 1) * 512],
                            lhsT=w2_sb[:, k * C + m * 128 : k * C + (m + 1) * 128],
                            rhs=hs[:, k * 512 : (k + 1) * 512],
                            start=(k == 0),
                            stop=(k == 1),
                        )
                col = qd * 512
                yv = bass.AP(
                    out_sb[:].tensor,
                    out_sb[:].offset + col,
                    [out_sb[:].ap[0], [NP, 2], [1, 512]],
                )
                if b2z:
                    if qd % 2 == 1:
                        nc.scalar.activation(out=yv, in_=py[:], func=AF.Copy)
                    else:
                        nc.vector.tensor_copy(yv, py[:])
                else:
                    for m in range(2):
                        nc.scalar.activation(
                            out=out_sb[:, m * NP + col : m * NP + col + 512],
                            in_=py[:, m * 512 : (m + 1) * 512],
                            func=AF.Identity,
                            bias=bia_sb[:, 2 + m : 3 + m],
                            scale=1.0,
                        )
                for m in range(2):
                    nc.sync.dma_start(
                        out=out[m, :, col : col + 512],
                        in_=out_sb[:, m * NP + col : m * NP + col + 512],
                    )
    return nc


def _pack_idx(idx_flat):
    """idx list (len = n*256, order: position j = blk*128+p) -> int16 SBUF
    layout [128, n*16]: entry j lives at (partition j%16, col j//16),
    replicated across the 8 groups of 16 partitions."""
    n = len(idx_flat) // 16
    arr = np.zeros((128, n), np.int16)
    block = np.asarray(idx_flat, np.int16).reshape(n, 16).T  # [16, n]
    for g in range(8):
        arr[g * 16 : (g + 1) * 16, :] = block
    return arr


def _make_in_maps(grid_features, station_coords, W1, b1, W2, b2):
    import jax
    import jax.numpy as jnp

    ix0, iy0, iy1, cjs = _host_route(station_coords)

    order0 = np.argsort(ix0, kind="stable")
    chunks = []
    los = []
    for c in range(NCORES):
        ch = order0[c * NPC : (c + 1) * NPC]
        ch = ch[np.argsort(iy0[ch], kind="stable")]
        chunks.append(ch)
        a = int(ix0[ch].min())
        b = int(ix0[ch].max())
        assert b - a + 2 <= WTBL, f"core {c} column spread {b - a} exceeds WTBL"
        los.append(a)

    # band cut positions in tiles
    band_t0 = np.cumsum([0] + BANDS_T).tolist()

    with jax.default_device(jax.devices("cpu")[0]):
        g = jnp.asarray(np.asarray(grid_features[0]))  # (C,H,W) f32
        gt = np.asarray(jnp.transpose(g, (1, 2, 0)).astype(jnp.bfloat16))  # (H,W,C)
        w1t = np.ascontiguousarray(
            np.asarray(jnp.asarray(np.asarray(W1, np.float32).T).astype(jnp.bfloat16))
        )
        w2t = np.ascontiguousarray(
            np.asarray(jnp.asarray(np.asarray(W2, np.float32).T).astype(jnp.bfloat16))
        )
        idn = np.asarray(
            jnp.asarray(np.tile(np.eye(128, dtype=np.float32), (1, 4))).astype(
                jnp.bfloat16
            )
        )
    bia = np.zeros((128, 4), np.float32)
    bia[:, 0] = b1[0:128]
    bia[:, 1] = b1[128:256]
    bia[:, 2] = b2[0:128]
    bia[:, 3] = b2[128:256]

    in_maps = []
    for c in range(NCORES):
        sids = chunks[c]
        cols = np.clip(np.arange(los[c], los[c] + WTBL), 0, W - 1)
        gtc = np.ascontiguousarray(gt[:, cols, :])  # (H, WTBL, C) bf16
        # 4-corner blocks: blk[y, x] = [g[y,x], g[y,x+1], g[y+1,x], g[y+1,x+1]]
        gp = np.concatenate([gtc, gtc[:, -1:, :]], axis=1)
        gp = np.concatenate([gp, gp[-1:, :, :]], axis=0)  # (H+1, WTBL+1, C)
        blk = np.concatenate(
            [gp[:-1, :-1], gp[:-1, 1:], gp[1:, :-1], gp[1:, 1:]], axis=2
        )  # (H, WTBL, 4C)
        x0l = (ix0[sids] - los[c]).astype(np.int64)
        tblc = np.zeros((TROWS, CB), gt.dtype)
        idx_parts = []
        for bnd in range(NBANDS):
            t0, t1 = band_t0[bnd], band_t0[bnd + 1]
            s0, s1 = t0 * 128, t1 * 128
            bids = np.arange(s0, s1)
            ybase = int(iy0[sids[s0]])
            yend = int(iy0[sids[s1 - 1]])
            nrow = (yend - ybase + 1) * WTBL
            assert nrow <= SLABROWS, f"band {bnd} rows {nrow} > {SLABROWS}"
            tblc[bnd * SLABROWS : bnd * SLABROWS + nrow] = blk[
                ybase : yend + 1
            ].reshape(nrow, CB)
            r0 = (iy0[sids[bids]] - ybase).astype(np.int64) * WTBL + x0l[bids]
            assert r0.max() <= MAX_IDX
            idx_parts.append(_pack_idx(r0))
        idx_arr = np.concatenate(idx_parts, axis=1)
        cof_t = np.stack(
            [cjs[j][sids].astype(np.float32).reshape(T, 128) for j in range(4)],
            axis=2,
        )  # [T, 128, 4] -> want [128, T*4] with col 4t+j
        cof_arr = np.asarray(
            jnp.asarray(
                np.ascontiguousarray(cof_t.transpose(1, 0, 2).reshape(128, 4 * T))
            ).astype(jnp.bfloat16)
        )
        in_maps.append(
            {
                "tbl": tblc,
                "idx": np.ascontiguousarray(idx_arr),
                "cof": cof_arr,
                "w1t": w1t,
                "w2t": w2t,
                "bia": bia,
                "idn": idn,
            }
        )
    return in_maps, chunks


def _install_ntff_shim():
    import sys
    import types

    try:
        import antenv.axon_hooks  # noqa: F401

        return
    except ImportError:
        pass
    from trn_agent_boot.trn_boot import _ntff_profile_via_ctypes

    hook = _ntff_profile_via_ctypes("/opt/axon/libaxon_pjrt.so")
    mod = types.ModuleType("antenv.axon_hooks")
    mod.get_axon_ntff_profile_hook = lambda: hook
    mod.set_axon_ntff_profile_hook = lambda h: None
    sys.modules["antenv.axon_hooks"] = mod


def _get_program(b1z=False, b2z=False):
    key = (b1z, b2z, bool(os.environ.get("GRIDSTN_NOGELU")))
    if key not in _PROG_CACHE:
        _PROG_CACHE[key] = _build_program(b1z, b2z)
    return _PROG_CACHE[key]


def kernel(grid_features, station_coords, W1, b1, W2, b2):
    in_maps, chunks = _make_in_maps(
        grid_features, station_coords, W1, b1, W2, b2
    )
    b1z = not np.any(np.asarray(b1))
    b2z = not np.any(np.asarray(b2))
    nc = _get_program(b1z, b2z)

    from concourse.bass_utils import run_bass_kernel_spmd

    trace = bool(os.environ.get("GRIDSTN_TRACE"))
    if trace:
        _install_ntff_shim()
    if not nc.is_finalized():
        nc.finalize()
    res = run_bass_kernel_spmd(nc, in_maps, list(range(NCORES)), trace=trace)
    LAST_RUN_INFO["exec_time_ns"] = res.exec_time_ns
    LAST_RUN_INFO["mean_exec_time_ns"] = res.mean_exec_time_ns
    LAST_RUN_INFO["profile_json"] = res.profile_json
    outs = [np.asarray(r["out"], np.float32) for r in res.results]

    result = np.zeros((N, C), np.float32)
    for c in range(NCORES):
        y = outs[c].reshape(2 * 128, NP)
        result[chunks[c]] = y.T
    return result.reshape(B, N, C)

